# revision 21
# baseline (speedup 1.0000x reference)
"""Trainium2 Bass kernel for nn_PatchMMConvolution.

Computes a shared-weight 3x3 conv (stride 1, pad 1) over x[B=2, P=18, Cin=64,
H=128, W=128] with weight[Cout=128, Cin=64, 3, 3] + bias, i.e. conv2d on
36 images, returning [2, 18, 128, 128, 128] float32.

Strategy (8 NeuronCores, SPMD single program):
  - 36 images are split into 16 "streams" of 288 output rows each
    (2 full images + one quarter-image per stream). Each core runs two
    streams: stream A in SBUF partitions 0-63, stream B in partitions 64-127
    (Cin=64 channels live on partitions).
  - Host pre-pads each stream into a "slab" [64, 294, 130]: three vertically
    concatenated zero-padded segments (130+130+34 rows, W padded to 130).
  - Conv is 9 shifted matmuls accumulating in PSUM: for each tap (kh,kw),
    lhsT = weight[kh,kw] as [Cin=64, Cout=128], rhs = shifted input window
    [64, 4 rows x 128 cols] (N=512). K=64 matmuls for streams A and B use
    PE row-groups 0-1 and 2-3 concurrently (tile_position derived from the
    base partition), so the two streams overlap on the PE array.
  - Bias is added during the PSUM->SBUF evacuation on the Vector engine.
"""

import numpy as np

import concourse.bass as bass
import concourse.mybir as mybir
import concourse.tile as tile
from concourse import bacc
from concourse._compat import get_trn_type
from concourse.bass_utils import run_bass_kernel_spmd
from concourse.tile_rust import add_dep_helper

B, PP, CIN, H, W = 2, 18, 64, 128, 128
COUT = 128
NIMG = B * PP  # 36
NCORES = 8
NSTREAM = 16
WP = W + 2  # 130 padded width
RSLAB = 294  # 130 + 130 + 34 slab rows per stream
ROWS_PER_STREAM = 288
# (slab_row_base, out_row_base, out_rows) per segment
SEGS = [(0, 0, 128), (130, 128, 128), (260, 256, 32)]
CHUNK_OUT_ROWS = 32  # output rows per input chunk
CHUNK_ROWS = CHUNK_OUT_ROWS + 2  # 34 input rows per chunk
TILE_OUT_ROWS = 4  # output rows per matmul tile (4*128 = 512 = one PSUM bank)

DT = mybir.dt.bfloat16  # matmul input dtype (quantized; rel err ~2.4e-3)
WEIGHT_DT = None  # optional override for the stationary (weight) dtype
ACC = mybir.dt.float32
OUT_DT = mybir.dt.bfloat16  # output DMA dtype (halves HBM write traffic)

# Benchmark knob: repeat the whole kernel body KERNEL_REPS times inside a
# hardware loop (used to isolate device exec time from dispatch overhead).
KERNEL_REPS = 1
# Subtractive-probe knobs (benchmarking only; break correctness when set).
SKIP_OUT_DMA = False
SKIP_EVAC = False
SKIP_IN_DMA = False
QUAD = False  # 4-way PE tiling (2 row groups x 2 col groups of M=64)
PSUM_BUFS = 4  # buffers per psum tag; with QUAD use 2 (4 tags -> 8 banks)
IN_BUFS = 4  # input chunk double-buffering depth
OUT_BUFS = 4  # output tile buffering depth
# Weight-stationary grouping: one full-array LDWEIGHTS per tap feeds GROUP_K
# tile-pairs of non-self-loading matmuls, amortizing the ~107ns weight load
# (which otherwise serializes with its row group's matmul stream).
GROUPED = True
GROUP_K = 4  # tile-pairs per weight-stationary group (2*GROUP_K PSUM banks)

_PROGRAM = None
EVAC_COUNTER = [0]


def _build_program():
    EVAC_COUNTER[0] = 0
    nc = bacc.Bacc(get_trn_type() or "TRN2", target_bir_lowering=False)
    wdt = WEIGHT_DT or DT
    xs = nc.dram_tensor("xs", [128, RSLAB, WP], DT, kind="ExternalInput")
    wd = nc.dram_tensor("wt", [128, 9, COUT], wdt, kind="ExternalInput")
    bd = nc.dram_tensor("bias", [COUT, 1], ACC, kind="ExternalInput")
    od = nc.dram_tensor(
        "out", [COUT, 2, ROWS_PER_STREAM, W], OUT_DT, kind="ExternalOutput"
    )

    chunks = []
    for sb, ob, nr in SEGS:
        for j in range(nr // CHUNK_OUT_ROWS):
            chunks.append((sb + CHUNK_OUT_ROWS * j, ob + CHUNK_OUT_ROWS * j))

    with tile.TileContext(nc) as tc:
        with (
            tc.tile_pool(name="const", bufs=1) as cpool,
            tc.tile_pool(name="inp", bufs=IN_BUFS) as ipool,
            tc.tile_pool(name="outp", bufs=OUT_BUFS) as opool,
            tc.tile_pool(name="ps", bufs=PSUM_BUFS, space="PSUM") as pspool,
        ):
            w_sb = cpool.tile([128, 9, COUT], wdt)
            nc.sync.dma_start(w_sb[:], wd[:])
            b_sb = cpool.tile([COUT, 1], ACC)
            nc.sync.dma_start(b_sb[:], bd[:])

            def emit_body_grouped():
                grp_rows = GROUP_K * TILE_OUT_ROWS
                # MMs issued since the last LDWEIGHTS; every new LDW takes an
                # ordering dep on them so the tile scheduler can never hoist a
                # weight load above matmuls that still need the old weights
                # (the PE itself never pulls a full-array LDW ahead).
                prev_mms = []
                for srow, orow in chunks:
                    ch = ipool.tile([128, CHUNK_ROWS, WP], DT, tag="chunk")
                    if not SKIP_IN_DMA:
                        nc.sync.dma_start(ch[:], xs[:, srow : srow + CHUNK_ROWS, :])
                    for g in range(CHUNK_OUT_ROWS // grp_rows):
                        g0 = grp_rows * g
                        ps = [
                            (
                                pspool.tile(
                                    [128, TILE_OUT_ROWS, W], ACC,
                                    tag="psA", name=f"psa{j}",
                                ),
                                pspool.tile(
                                    [128, TILE_OUT_ROWS, W], ACC,
                                    tag="psB", name=f"psb{j}",
                                ),
                            )
                            for j in range(GROUP_K)
                        ]
                        for tap in range(9):
                            kh, kw = divmod(tap, 3)
                            first, last = tap == 0, tap == 8
                            lw = nc.tensor.ldweights(w_sb[:, tap])
                            for pm in prev_mms:
                                add_dep_helper(lw.ins, pm.ins, False, "ldw after prev tap mms")
                            prev_mms.clear()
                            for j in range(GROUP_K):
                                r0 = g0 + TILE_OUT_ROWS * j
                                ra = ch[0:64, r0 + kh : r0 + kh + TILE_OUT_ROWS, kw : kw + W]
                                rb = ch[64:128, r0 + kh : r0 + kh + TILE_OUT_ROWS, kw : kw + W]
                                ma = nc.tensor.matmul(
                                    ps[j][0][:], w_sb[0:64, tap], ra,
                                    start=first, stop=last)
                                mb = nc.tensor.matmul(
                                    ps[j][1][:], w_sb[64:128, tap], rb,
                                    start=first, stop=last)
                                ma.ins.ldweights = False
                                mb.ins.ldweights = False
                                add_dep_helper(ma.ins, lw.ins, False, "mm after ldw")
                                add_dep_helper(mb.ins, lw.ins, False, "mm after ldw")
                                prev_mms += [ma, mb]
                        if SKIP_EVAC:
                            continue
                        stage = opool.tile([128, 2, grp_rows, W], OUT_DT, tag="stage")
                        for j in range(GROUP_K):
                            sl = slice(TILE_OUT_ROWS * j, TILE_OUT_ROWS * (j + 1))
                            nc.vector.tensor_scalar_add(
                                stage[:, 0, sl], ps[j][0][:], b_sb[:])
                            nc.scalar.add(stage[:, 1, sl], ps[j][1][:], b_sb[:])
                        if SKIP_OUT_DMA:
                            continue
                        orr = orow + g0
                        nc.sync.dma_start(
                            od[:, 0, orr : orr + grp_rows, :], stage[:, 0])
                        nc.sync.dma_start(
                            od[:, 1, orr : orr + grp_rows, :], stage[:, 1])

            def emit_body():
                for srow, orow in chunks:
                    ch = ipool.tile([128, CHUNK_ROWS, WP], DT, tag="chunk")
                    if not SKIP_IN_DMA:
                        nc.sync.dma_start(ch[:], xs[:, srow : srow + CHUNK_ROWS, :])
                    for i in range(CHUNK_OUT_ROWS // TILE_OUT_ROWS):
                        psa = pspool.tile([128, TILE_OUT_ROWS, W], ACC, tag="psA")
                        psb = pspool.tile([128, TILE_OUT_ROWS, W], ACC, tag="psB")
                        r0 = TILE_OUT_ROWS * i
                        if QUAD:
                            psa2 = pspool.tile(
                                [128, TILE_OUT_ROWS, W], ACC, tag="psA2"
                            )
                            psb2 = pspool.tile(
                                [128, TILE_OUT_ROWS, W], ACC, tag="psB2"
                            )
                        for tap in range(9):
                            kh, kw = divmod(tap, 3)
                            first, last = tap == 0, tap == 8
                            ra = ch[0:64, r0 + kh : r0 + kh + TILE_OUT_ROWS, kw : kw + W]
                            rb = ch[64:128, r0 + kh : r0 + kh + TILE_OUT_ROWS, kw : kw + W]
                            if QUAD:
                                nc.tensor.matmul(
                                    psa[0:64], w_sb[0:64, tap, 0:64], ra,
                                    start=first, stop=last)
                                nc.tensor.matmul(
                                    psb[0:64], w_sb[64:128, tap, 0:64], rb,
                                    start=first, stop=last)
                                nc.tensor.matmul(
                                    psa2[64:128], w_sb[0:64, tap, 64:128], ra,
                                    start=first, stop=last)
                                nc.tensor.matmul(
                                    psb2[64:128], w_sb[64:128, tap, 64:128], rb,
                                    start=first, stop=last)
                            else:
                                nc.tensor.matmul(
                                    psa[:], w_sb[0:64, tap], ra,
                                    start=first, stop=last)
                                nc.tensor.matmul(
                                    psb[:], w_sb[64:128, tap], rb,
                                    start=first, stop=last)
                        if SKIP_EVAC:
                            continue
                        oa = opool.tile([128, TILE_OUT_ROWS, W], OUT_DT, tag="oA")
                        obt = opool.tile([128, TILE_OUT_ROWS, W], OUT_DT, tag="oB")
                        if QUAD:
                            nc.vector.tensor_scalar_add(
                                oa[0:64], psa[0:64], b_sb[0:64])
                            nc.vector.tensor_scalar_add(
                                oa[64:128], psa2[64:128], b_sb[64:128])
                            nc.vector.tensor_scalar_add(
                                obt[0:64], psb[0:64], b_sb[0:64])
                            nc.vector.tensor_scalar_add(
                                obt[64:128], psb2[64:128], b_sb[64:128])
                        else:
                            # PSUM->SBUF evacuation (+bias) split between
                            # the two PSUM-capable elementwise engines
                            # (GPSIMD/Pool cannot read PSUM on TRN2).
                            nc.vector.tensor_scalar_add(oa[:], psa[:], b_sb[:])
                            nc.scalar.add(obt[:], psb[:], b_sb[:])
                        if SKIP_OUT_DMA:
                            continue
                        orr = orow + r0
                        nc.sync.dma_start(
                            od[:, 0, orr : orr + TILE_OUT_ROWS, :], oa[:]
                        )
                        nc.sync.dma_start(
                            od[:, 1, orr : orr + TILE_OUT_ROWS, :], obt[:]
                        )

            body = emit_body_grouped if GROUPED else emit_body
            if KERNEL_REPS > 1:
                with tc.For_i(0, KERNEL_REPS, 1) as _i:
                    body()
            else:
                body()
    nc.finalize()
    return nc


def _get_program():
    global _PROGRAM
    if _PROGRAM is None:
        _PROGRAM = _build_program()
    return _PROGRAM


def _stream_parts(s):
    """Stream s covers full images 2s, 2s+1 and quarter (s%4) of image 32+(s//4)...
    returns (img0, img1, img_q, q) with quarter rows [32q, 32q+32)."""
    img_q = 32 + (s % 4)
    q = s // 4
    return 2 * s, 2 * s + 1, img_q, q


def _make_slab(X, s):
    """Build padded slab [CIN, RSLAB, WP] for stream s from X [NIMG,CIN,H,W]."""
    i0, i1, iq, q = _stream_parts(s)
    sl = np.zeros((CIN, RSLAB, WP), np.float32)
    sl[:, 1 : H + 1, 1 : W + 1] = X[i0]
    sl[:, 131 : 131 + H, 1 : W + 1] = X[i1]
    r0 = 32 * q
    lo, hi = max(r0 - 1, 0), min(r0 + 33, H)
    d0 = 260 + (lo - (r0 - 1))
    sl[:, d0 : d0 + (hi - lo), 1 : W + 1] = X[iq, :, lo:hi]
    return sl


def make_in_maps(x, weight, bias):
    x = np.ascontiguousarray(np.asarray(x), dtype=np.float32)
    weight = np.ascontiguousarray(np.asarray(weight), dtype=np.float32)
    bias = np.ascontiguousarray(np.asarray(bias), dtype=np.float32)
    X = x.reshape(NIMG, CIN, H, W)

    wt = np.ascontiguousarray(weight.transpose(1, 2, 3, 0).reshape(CIN, 9, COUT))
    wt2 = np.ascontiguousarray(np.concatenate([wt, wt], axis=0))  # [128, 9, COUT]
    wt2 = wt2.astype(mybir.dt.np(WEIGHT_DT or DT))
    bb = np.ascontiguousarray(bias.reshape(COUT, 1))

    in_dt = mybir.dt.np(DT)
    in_maps = []
    for c in range(NCORES):
        xs = np.concatenate([_make_slab(X, 2 * c), _make_slab(X, 2 * c + 1)], axis=0)
        in_maps.append(
            {"xs": np.ascontiguousarray(xs.astype(in_dt)), "wt": wt2, "bias": bb}
        )
    return in_maps


def kernel(x, weight, bias):
    in_maps = make_in_maps(x, weight, bias)
    nc = _get_program()
    res = run_bass_kernel_spmd(nc, in_maps, core_ids=list(range(NCORES)))

    Y = np.empty((NIMG, COUT, H, W), np.float32)
    for c in range(NCORES):
        o = res.results[c]["out"].astype(np.float32)  # [COUT, 2, 288, W]
        for half in (0, 1):
            s = 2 * c + half
            i0, i1, iq, q = _stream_parts(s)
            oo = o[:, half]
            Y[i0] = oo[:, 0:H]
            Y[i1] = oo[:, H : 2 * H]
            Y[iq, :, 32 * q : 32 * q + 32, :] = oo[:, 2 * H : 2 * H + 32]
    return Y.reshape(B, PP, COUT, H, W)



# revision 28
# speedup vs baseline: 1.2662x; 1.2662x over previous
"""Trainium2 Bass kernel for nn_PatchMMConvolution.

Computes a shared-weight 3x3 conv (stride 1, pad 1) over x[B=2, P=18, Cin=64,
H=128, W=128] with weight[Cout=128, Cin=64, 3, 3] + bias, i.e. conv2d on
36 images, returning [2, 18, 128, 128, 128] float32.

Strategy (8 NeuronCores, SPMD single program):
  - 36 images are split into 16 "streams" of 288 output rows each
    (2 full images + one quarter-image per stream). Each core runs two
    streams: stream A in SBUF partitions 0-63, stream B in partitions 64-127
    (Cin=64 channels live on partitions).
  - Host pre-pads each stream into a "slab" [64, 294, 130]: three vertically
    concatenated zero-padded segments (130+130+34 rows, W padded to 130).
  - Conv is 9 shifted matmuls accumulating in PSUM: for each tap (kh,kw),
    lhsT = weight[kh,kw] as [Cin=64, Cout=128], rhs = shifted input window
    [64, 4 rows x 128 cols] (N=512). K=64 matmuls for streams A and B use
    PE row-groups 0-1 and 2-3 concurrently (tile_position derived from the
    base partition), so the two streams overlap on the PE array.
  - Bias is added during the PSUM->SBUF evacuation on the Vector engine.
"""

import numpy as np

import concourse.bass as bass
import concourse.mybir as mybir
import concourse.tile as tile
from concourse import bacc
from concourse._compat import get_trn_type
from concourse.bass_utils import run_bass_kernel_spmd
from concourse.tile_rust import add_dep_helper

B, PP, CIN, H, W = 2, 18, 64, 128, 128
COUT = 128
NIMG = B * PP  # 36
NCORES = 8
NSTREAM = 16
WP = W + 2  # 130 padded width
RSLAB = 294  # 130 + 130 + 34 slab rows per stream
ROWS_PER_STREAM = 288
# (slab_row_base, out_row_base, out_rows) per segment
SEGS = [(0, 0, 128), (130, 128, 128), (260, 256, 32)]
CHUNK_OUT_ROWS = 32  # output rows per input chunk
CHUNK_ROWS = CHUNK_OUT_ROWS + 2  # 34 input rows per chunk
TILE_OUT_ROWS = 4  # output rows per matmul tile (4*128 = 512 = one PSUM bank)

DT = mybir.dt.bfloat16  # matmul input dtype (quantized; rel err ~2.4e-3)
WEIGHT_DT = None  # optional override for the stationary (weight) dtype
ACC = mybir.dt.float32
OUT_DT = mybir.dt.bfloat16  # output DMA dtype (halves HBM write traffic)

# Benchmark knob: repeat the whole kernel body KERNEL_REPS times inside a
# hardware loop (used to isolate device exec time from dispatch overhead).
KERNEL_REPS = 1
# Subtractive-probe knobs (benchmarking only; break correctness when set).
SKIP_OUT_DMA = False
SKIP_EVAC = False
SKIP_IN_DMA = False
QUAD = False  # 4-way PE tiling (2 row groups x 2 col groups of M=64)
PSUM_BUFS = 4  # buffers per psum tag; with QUAD use 2 (4 tags -> 8 banks)
IN_BUFS = 4  # input chunk double-buffering depth
OUT_BUFS = 4  # output tile buffering depth
# Weight-stationary grouping: one full-array LDWEIGHTS per tap feeds GROUP_K
# tile-pairs of non-self-loading matmuls, amortizing the ~107ns weight load
# (which otherwise serializes with its row group's matmul stream).
# Weight-stationary grouping with explicit LDWEIGHTS: measured SLOWER than
# the self-loading interleaved A/B stream on hardware (216-238us vs 182us),
# so disabled. The self-loading stream runs at ~270ns per 512-element slot,
# matching the HW-measured production roofline (~131ns/MM at N=512).
GROUPED = False
GROUP_K = 3
# Batch PSUM evacuations of a whole chunk into one SBUF staging tile and
# write it out with one DMA per stream (18 output DMAs/rep instead of 288).
STAGED_OUT = True

_PROGRAM = None
EVAC_COUNTER = [0]


def _build_program():
    EVAC_COUNTER[0] = 0
    nc = bacc.Bacc(get_trn_type() or "TRN2", target_bir_lowering=False)
    wdt = WEIGHT_DT or DT
    xs = nc.dram_tensor("xs", [128, RSLAB, WP], DT, kind="ExternalInput")
    wd = nc.dram_tensor("wt", [128, 9, COUT], wdt, kind="ExternalInput")
    bd = nc.dram_tensor("bias", [COUT, 1], ACC, kind="ExternalInput")
    od = nc.dram_tensor(
        "out", [COUT, 2, ROWS_PER_STREAM, W], OUT_DT, kind="ExternalOutput"
    )

    chunks = []
    for sb, ob, nr in SEGS:
        for j in range(nr // CHUNK_OUT_ROWS):
            chunks.append((sb + CHUNK_OUT_ROWS * j, ob + CHUNK_OUT_ROWS * j))

    with tile.TileContext(nc) as tc:
        with (
            tc.tile_pool(name="const", bufs=1) as cpool,
            tc.tile_pool(name="inp", bufs=IN_BUFS) as ipool,
            tc.tile_pool(name="outp", bufs=OUT_BUFS) as opool,
            tc.tile_pool(name="ps", bufs=PSUM_BUFS, space="PSUM") as pspool,
        ):
            w_sb = cpool.tile([128, 9, COUT], wdt)
            nc.sync.dma_start(w_sb[:], wd[:])
            b_sb = cpool.tile([COUT, 1], ACC)
            nc.sync.dma_start(b_sb[:], bd[:])

            def emit_body_grouped():
                # MMs issued since the last LDWEIGHTS; every new LDW takes an
                # ordering dep on them so the tile scheduler can never hoist a
                # weight load above matmuls that still need the old weights
                # (the PE itself never pulls a full-array LDW ahead).
                prev_mms = []
                npairs = CHUNK_OUT_ROWS // TILE_OUT_ROWS
                for srow, orow in chunks:
                    ch = ipool.tile([128, CHUNK_ROWS, WP], DT, tag="chunk")
                    if not SKIP_IN_DMA:
                        nc.sync.dma_start(ch[:], xs[:, srow : srow + CHUNK_ROWS, :])
                    for p0 in range(0, npairs, GROUP_K):
                        k = min(GROUP_K, npairs - p0)
                        g0 = TILE_OUT_ROWS * p0
                        grp_rows = TILE_OUT_ROWS * k
                        ps = [
                            (
                                pspool.tile(
                                    [128, TILE_OUT_ROWS, W], ACC,
                                    tag="psA", name=f"psa{j}",
                                ),
                                pspool.tile(
                                    [128, TILE_OUT_ROWS, W], ACC,
                                    tag="psB", name=f"psb{j}",
                                ),
                            )
                            for j in range(k)
                        ]
                        for tap in range(9):
                            kh, kw = divmod(tap, 3)
                            first, last = tap == 0, tap == 8
                            lw = nc.tensor.ldweights(w_sb[:, tap])
                            for pm in prev_mms:
                                add_dep_helper(lw.ins, pm.ins, False, "ldw after prev tap mms")
                            prev_mms.clear()
                            for j in range(k):
                                r0 = g0 + TILE_OUT_ROWS * j
                                ra = ch[0:64, r0 + kh : r0 + kh + TILE_OUT_ROWS, kw : kw + W]
                                rb = ch[64:128, r0 + kh : r0 + kh + TILE_OUT_ROWS, kw : kw + W]
                                ma = nc.tensor.matmul(
                                    ps[j][0][:], w_sb[0:64, tap], ra,
                                    start=first, stop=last)
                                mb = nc.tensor.matmul(
                                    ps[j][1][:], w_sb[64:128, tap], rb,
                                    start=first, stop=last)
                                ma.ins.ldweights = False
                                mb.ins.ldweights = False
                                add_dep_helper(ma.ins, lw.ins, False, "mm after ldw")
                                add_dep_helper(mb.ins, lw.ins, False, "mm after ldw")
                                prev_mms += [ma, mb]
                        if SKIP_EVAC:
                            continue
                        stage = opool.tile(
                            [128, 2, grp_rows, W], OUT_DT, tag="stage", name="stage"
                        )
                        for j in range(k):
                            sl = slice(TILE_OUT_ROWS * j, TILE_OUT_ROWS * (j + 1))
                            nc.vector.tensor_scalar_add(
                                stage[:, 0, sl], ps[j][0][:], b_sb[:])
                            nc.scalar.add(stage[:, 1, sl], ps[j][1][:], b_sb[:])
                        if SKIP_OUT_DMA:
                            continue
                        orr = orow + g0
                        nc.sync.dma_start(
                            od[:, 0, orr : orr + grp_rows, :], stage[:, 0])
                        nc.sync.dma_start(
                            od[:, 1, orr : orr + grp_rows, :], stage[:, 1])

            def emit_body():
                for srow, orow in chunks:
                    ch = ipool.tile([128, CHUNK_ROWS, WP], DT, tag="chunk")
                    if not SKIP_IN_DMA:
                        nc.sync.dma_start(ch[:], xs[:, srow : srow + CHUNK_ROWS, :])
                    stage = None
                    if STAGED_OUT and not SKIP_EVAC:
                        stage = opool.tile(
                            [128, 2, CHUNK_OUT_ROWS, W], OUT_DT,
                            tag="stage", name="stage",
                        )
                    for i in range(CHUNK_OUT_ROWS // TILE_OUT_ROWS):
                        psa = pspool.tile([128, TILE_OUT_ROWS, W], ACC, tag="psA")
                        psb = pspool.tile([128, TILE_OUT_ROWS, W], ACC, tag="psB")
                        r0 = TILE_OUT_ROWS * i
                        if QUAD:
                            psa2 = pspool.tile(
                                [128, TILE_OUT_ROWS, W], ACC, tag="psA2"
                            )
                            psb2 = pspool.tile(
                                [128, TILE_OUT_ROWS, W], ACC, tag="psB2"
                            )
                        for tap in range(9):
                            kh, kw = divmod(tap, 3)
                            first, last = tap == 0, tap == 8
                            ra = ch[0:64, r0 + kh : r0 + kh + TILE_OUT_ROWS, kw : kw + W]
                            rb = ch[64:128, r0 + kh : r0 + kh + TILE_OUT_ROWS, kw : kw + W]
                            if QUAD:
                                nc.tensor.matmul(
                                    psa[0:64], w_sb[0:64, tap, 0:64], ra,
                                    start=first, stop=last)
                                nc.tensor.matmul(
                                    psb[0:64], w_sb[64:128, tap, 0:64], rb,
                                    start=first, stop=last)
                                nc.tensor.matmul(
                                    psa2[64:128], w_sb[0:64, tap, 64:128], ra,
                                    start=first, stop=last)
                                nc.tensor.matmul(
                                    psb2[64:128], w_sb[64:128, tap, 64:128], rb,
                                    start=first, stop=last)
                            else:
                                nc.tensor.matmul(
                                    psa[:], w_sb[0:64, tap], ra,
                                    start=first, stop=last)
                                nc.tensor.matmul(
                                    psb[:], w_sb[64:128, tap], rb,
                                    start=first, stop=last)
                        if SKIP_EVAC:
                            continue
                        if STAGED_OUT:
                            sl = slice(r0, r0 + TILE_OUT_ROWS)
                            nc.vector.tensor_scalar_add(
                                stage[:, 0, sl], psa[:], b_sb[:])
                            nc.scalar.add(stage[:, 1, sl], psb[:], b_sb[:])
                            continue
                        oa = opool.tile([128, TILE_OUT_ROWS, W], OUT_DT, tag="oA")
                        obt = opool.tile([128, TILE_OUT_ROWS, W], OUT_DT, tag="oB")
                        if QUAD:
                            nc.vector.tensor_scalar_add(
                                oa[0:64], psa[0:64], b_sb[0:64])
                            nc.vector.tensor_scalar_add(
                                oa[64:128], psa2[64:128], b_sb[64:128])
                            nc.vector.tensor_scalar_add(
                                obt[0:64], psb[0:64], b_sb[0:64])
                            nc.vector.tensor_scalar_add(
                                obt[64:128], psb2[64:128], b_sb[64:128])
                        else:
                            # PSUM->SBUF evacuation (+bias) split between
                            # the two PSUM-capable elementwise engines
                            # (GPSIMD/Pool cannot read PSUM on TRN2).
                            nc.vector.tensor_scalar_add(oa[:], psa[:], b_sb[:])
                            nc.scalar.add(obt[:], psb[:], b_sb[:])
                        if SKIP_OUT_DMA:
                            continue
                        orr = orow + r0
                        nc.sync.dma_start(
                            od[:, 0, orr : orr + TILE_OUT_ROWS, :], oa[:]
                        )
                        nc.sync.dma_start(
                            od[:, 1, orr : orr + TILE_OUT_ROWS, :], obt[:]
                        )
                    if STAGED_OUT and not SKIP_EVAC and not SKIP_OUT_DMA:
                        nc.sync.dma_start(
                            od[:, 0, orow : orow + CHUNK_OUT_ROWS, :], stage[:, 0]
                        )
                        nc.sync.dma_start(
                            od[:, 1, orow : orow + CHUNK_OUT_ROWS, :], stage[:, 1]
                        )

            body = emit_body_grouped if GROUPED else emit_body
            if KERNEL_REPS > 1:
                with tc.For_i(0, KERNEL_REPS, 1) as _i:
                    body()
            else:
                body()
    nc.finalize()
    return nc


def _get_program():
    global _PROGRAM
    if _PROGRAM is None:
        _PROGRAM = _build_program()
    return _PROGRAM


def _stream_parts(s):
    """Stream s covers full images 2s, 2s+1 and quarter (s%4) of image 32+(s//4)...
    returns (img0, img1, img_q, q) with quarter rows [32q, 32q+32)."""
    img_q = 32 + (s % 4)
    q = s // 4
    return 2 * s, 2 * s + 1, img_q, q


def _make_slab(X, s):
    """Build padded slab [CIN, RSLAB, WP] for stream s from X [NIMG,CIN,H,W]."""
    i0, i1, iq, q = _stream_parts(s)
    sl = np.zeros((CIN, RSLAB, WP), np.float32)
    sl[:, 1 : H + 1, 1 : W + 1] = X[i0]
    sl[:, 131 : 131 + H, 1 : W + 1] = X[i1]
    r0 = 32 * q
    lo, hi = max(r0 - 1, 0), min(r0 + 33, H)
    d0 = 260 + (lo - (r0 - 1))
    sl[:, d0 : d0 + (hi - lo), 1 : W + 1] = X[iq, :, lo:hi]
    return sl


def make_in_maps(x, weight, bias):
    x = np.ascontiguousarray(np.asarray(x), dtype=np.float32)
    weight = np.ascontiguousarray(np.asarray(weight), dtype=np.float32)
    bias = np.ascontiguousarray(np.asarray(bias), dtype=np.float32)
    X = x.reshape(NIMG, CIN, H, W)

    wt = np.ascontiguousarray(weight.transpose(1, 2, 3, 0).reshape(CIN, 9, COUT))
    wt2 = np.ascontiguousarray(np.concatenate([wt, wt], axis=0))  # [128, 9, COUT]
    wt2 = wt2.astype(mybir.dt.np(WEIGHT_DT or DT))
    bb = np.ascontiguousarray(bias.reshape(COUT, 1))

    in_dt = mybir.dt.np(DT)
    in_maps = []
    for c in range(NCORES):
        xs = np.concatenate([_make_slab(X, 2 * c), _make_slab(X, 2 * c + 1)], axis=0)
        in_maps.append(
            {"xs": np.ascontiguousarray(xs.astype(in_dt)), "wt": wt2, "bias": bb}
        )
    return in_maps


def kernel(x, weight, bias):
    in_maps = make_in_maps(x, weight, bias)
    nc = _get_program()
    res = run_bass_kernel_spmd(nc, in_maps, core_ids=list(range(NCORES)))

    Y = np.empty((NIMG, COUT, H, W), np.float32)
    for c in range(NCORES):
        o = res.results[c]["out"].astype(np.float32)  # [COUT, 2, 288, W]
        for half in (0, 1):
            s = 2 * c + half
            i0, i1, iq, q = _stream_parts(s)
            oo = o[:, half]
            Y[i0] = oo[:, 0:H]
            Y[i1] = oo[:, H : 2 * H]
            Y[iq, :, 32 * q : 32 * q + 32, :] = oo[:, 2 * H : 2 * H + 32]
    return Y.reshape(B, PP, COUT, H, W)



# revision 32
# speedup vs baseline: 1.3138x; 1.0376x over previous
"""Trainium2 Bass kernel for nn_PatchMMConvolution.

Computes a shared-weight 3x3 conv (stride 1, pad 1) over x[B=2, P=18, Cin=64,
H=128, W=128] with weight[Cout=128, Cin=64, 3, 3] + bias, i.e. conv2d on
36 images, returning [2, 18, 128, 128, 128] float32.

Strategy (8 NeuronCores, SPMD single program):
  - 36 images are split into 16 "streams" of 288 output rows each
    (2 full images + one quarter-image per stream). Each core runs two
    streams: stream A in SBUF partitions 0-63, stream B in partitions 64-127
    (Cin=64 channels live on partitions).
  - Host pre-pads each stream into a "slab" [64, 294, 130] in bfloat16:
    three vertically concatenated zero-padded segments (130+130+34 rows,
    W padded to 130). bf16 halves HBM traffic vs fp32 (in 9.8MB + out
    18.9MB per core ~= 80us at 360GB/s, hidden under compute) at
    ~2.9e-3 relative error; PSUM accumulation stays fp32.
  - Conv is 9 shifted matmuls accumulating in PSUM: for each tap (kh,kw),
    lhsT = weight[kh,kw] as [Cin=64, Cout=128], rhs = shifted input window
    [64, 4 rows x 128 cols] (N=512). Self-loading K=64 matmuls for streams
    A and B use PE row-groups 0-1 and 2-3 concurrently (tile_position from
    the base partition). The measured slot rate (~270ns per 512-cycle
    pair) matches the HW production LDWEIGHTS+MATMUL roofline; explicit
    weight-stationary LDW grouping measured slower (see GROUPED).
  - PSUM evacuation (+bias, fp32->bf16) is split between the Vector and
    Scalar engines (GPSIMD cannot read PSUM), staged per chunk in SBUF,
    and written out with one DMA per stream per chunk.
"""

import numpy as np

import concourse.bass as bass
import concourse.mybir as mybir
import concourse.tile as tile
from concourse import bacc
from concourse._compat import get_trn_type
from concourse.bass_utils import run_bass_kernel_spmd
from concourse.tile_rust import add_dep_helper

B, PP, CIN, H, W = 2, 18, 64, 128, 128
COUT = 128
NIMG = B * PP  # 36
NCORES = 8
NSTREAM = 16
WP = W + 2  # 130 padded width
RSLAB = 294  # 130 + 130 + 34 slab rows per stream
ROWS_PER_STREAM = 288
# (slab_row_base, out_row_base, out_rows) per segment
SEGS = [(0, 0, 128), (130, 128, 128), (260, 256, 32)]
CHUNK_OUT_ROWS = 32  # output rows per input chunk
CHUNK_ROWS = CHUNK_OUT_ROWS + 2  # 34 input rows per chunk
TILE_OUT_ROWS = 4  # output rows per matmul tile (4*128 = 512 = one PSUM bank)

DT = mybir.dt.bfloat16  # matmul input dtype (quantized; rel err ~2.4e-3)
WEIGHT_DT = None  # optional override for the stationary (weight) dtype
ACC = mybir.dt.float32
OUT_DT = mybir.dt.bfloat16  # output DMA dtype (halves HBM write traffic)

# Benchmark knob: repeat the whole kernel body KERNEL_REPS times inside a
# hardware loop (used to isolate device exec time from dispatch overhead).
KERNEL_REPS = 1
# Subtractive-probe knobs (benchmarking only; break correctness when set).
SKIP_OUT_DMA = False
SKIP_EVAC = False
SKIP_IN_DMA = False
QUAD = False  # 4-way PE tiling (2 row groups x 2 col groups of M=64)
PSUM_BUFS = 4  # buffers per psum tag; with QUAD use 2 (4 tags -> 8 banks)
IN_BUFS = 4  # input chunk double-buffering depth
OUT_BUFS = 4  # output tile buffering depth
# Weight-stationary grouping: one full-array LDWEIGHTS per tap feeds GROUP_K
# tile-pairs of non-self-loading matmuls, amortizing the ~107ns weight load
# (which otherwise serializes with its row group's matmul stream).
# Weight-stationary grouping with explicit LDWEIGHTS: measured SLOWER than
# the self-loading interleaved A/B stream on hardware (216-238us vs 182us),
# so disabled. The self-loading stream runs at ~270ns per 512-element slot,
# matching the HW-measured production roofline (~131ns/MM at N=512).
GROUPED = False
GROUP_K = 3
# Batch PSUM evacuations of a whole chunk into one SBUF staging tile and
# write it out with one DMA per stream (18 output DMAs/rep instead of 288).
STAGED_OUT = True

_PROGRAM = None
EVAC_COUNTER = [0]


def _build_program():
    EVAC_COUNTER[0] = 0
    nc = bacc.Bacc(get_trn_type() or "TRN2", target_bir_lowering=False)
    wdt = WEIGHT_DT or DT
    xs = nc.dram_tensor("xs", [128, RSLAB, WP], DT, kind="ExternalInput")
    wd = nc.dram_tensor("wt", [128, 9, COUT], wdt, kind="ExternalInput")
    bd = nc.dram_tensor("bias", [COUT, 1], ACC, kind="ExternalInput")
    od = nc.dram_tensor(
        "out", [COUT, 2, ROWS_PER_STREAM, W], OUT_DT, kind="ExternalOutput"
    )

    chunks = []
    for sb, ob, nr in SEGS:
        for j in range(nr // CHUNK_OUT_ROWS):
            chunks.append((sb + CHUNK_OUT_ROWS * j, ob + CHUNK_OUT_ROWS * j))

    with tile.TileContext(nc) as tc:
        with (
            tc.tile_pool(name="const", bufs=1) as cpool,
            tc.tile_pool(name="inp", bufs=IN_BUFS) as ipool,
            tc.tile_pool(name="outp", bufs=OUT_BUFS) as opool,
            tc.tile_pool(name="ps", bufs=PSUM_BUFS, space="PSUM") as pspool,
        ):
            w_sb = cpool.tile([128, 9, COUT], wdt)
            nc.sync.dma_start(w_sb[:], wd[:])
            b_sb = cpool.tile([COUT, 1], ACC)
            nc.sync.dma_start(b_sb[:], bd[:])

            def emit_body_grouped():
                # MMs issued since the last LDWEIGHTS; every new LDW takes an
                # ordering dep on them so the tile scheduler can never hoist a
                # weight load above matmuls that still need the old weights
                # (the PE itself never pulls a full-array LDW ahead).
                prev_mms = []
                npairs = CHUNK_OUT_ROWS // TILE_OUT_ROWS
                for srow, orow in chunks:
                    ch = ipool.tile([128, CHUNK_ROWS, WP], DT, tag="chunk")
                    if not SKIP_IN_DMA:
                        nc.sync.dma_start(ch[:], xs[:, srow : srow + CHUNK_ROWS, :])
                    for p0 in range(0, npairs, GROUP_K):
                        k = min(GROUP_K, npairs - p0)
                        g0 = TILE_OUT_ROWS * p0
                        grp_rows = TILE_OUT_ROWS * k
                        ps = [
                            (
                                pspool.tile(
                                    [128, TILE_OUT_ROWS, W], ACC,
                                    tag="psA", name=f"psa{j}",
                                ),
                                pspool.tile(
                                    [128, TILE_OUT_ROWS, W], ACC,
                                    tag="psB", name=f"psb{j}",
                                ),
                            )
                            for j in range(k)
                        ]
                        for tap in range(9):
                            kh, kw = divmod(tap, 3)
                            first, last = tap == 0, tap == 8
                            lw = nc.tensor.ldweights(w_sb[:, tap])
                            for pm in prev_mms:
                                add_dep_helper(lw.ins, pm.ins, False, "ldw after prev tap mms")
                            prev_mms.clear()
                            for j in range(k):
                                r0 = g0 + TILE_OUT_ROWS * j
                                ra = ch[0:64, r0 + kh : r0 + kh + TILE_OUT_ROWS, kw : kw + W]
                                rb = ch[64:128, r0 + kh : r0 + kh + TILE_OUT_ROWS, kw : kw + W]
                                ma = nc.tensor.matmul(
                                    ps[j][0][:], w_sb[0:64, tap], ra,
                                    start=first, stop=last)
                                mb = nc.tensor.matmul(
                                    ps[j][1][:], w_sb[64:128, tap], rb,
                                    start=first, stop=last)
                                ma.ins.ldweights = False
                                mb.ins.ldweights = False
                                add_dep_helper(ma.ins, lw.ins, False, "mm after ldw")
                                add_dep_helper(mb.ins, lw.ins, False, "mm after ldw")
                                prev_mms += [ma, mb]
                        if SKIP_EVAC:
                            continue
                        stage = opool.tile(
                            [128, 2, grp_rows, W], OUT_DT, tag="stage", name="stage"
                        )
                        for j in range(k):
                            sl = slice(TILE_OUT_ROWS * j, TILE_OUT_ROWS * (j + 1))
                            nc.vector.tensor_scalar_add(
                                stage[:, 0, sl], ps[j][0][:], b_sb[:])
                            nc.scalar.add(stage[:, 1, sl], ps[j][1][:], b_sb[:])
                        if SKIP_OUT_DMA:
                            continue
                        orr = orow + g0
                        nc.sync.dma_start(
                            od[:, 0, orr : orr + grp_rows, :], stage[:, 0])
                        nc.sync.dma_start(
                            od[:, 1, orr : orr + grp_rows, :], stage[:, 1])

            def emit_body():
                for srow, orow in chunks:
                    ch = ipool.tile([128, CHUNK_ROWS, WP], DT, tag="chunk")
                    if not SKIP_IN_DMA:
                        # Split the chunk load so the first tile-pair's rows
                        # (0..5) land early; subtile deps let its matmuls
                        # start while the rest of the chunk streams in.
                        # Only matters for single-shot head latency.
                        nc.sync.dma_start(ch[:, 0:6], xs[:, srow : srow + 6, :])
                        nc.sync.dma_start(
                            ch[:, 6:CHUNK_ROWS],
                            xs[:, srow + 6 : srow + CHUNK_ROWS, :],
                        )
                    stage = None
                    if STAGED_OUT and not SKIP_EVAC:
                        stage = opool.tile(
                            [128, 2, CHUNK_OUT_ROWS, W], OUT_DT,
                            tag="stage", name="stage",
                        )
                    for i in range(CHUNK_OUT_ROWS // TILE_OUT_ROWS):
                        psa = pspool.tile([128, TILE_OUT_ROWS, W], ACC, tag="psA")
                        psb = pspool.tile([128, TILE_OUT_ROWS, W], ACC, tag="psB")
                        r0 = TILE_OUT_ROWS * i
                        if QUAD:
                            psa2 = pspool.tile(
                                [128, TILE_OUT_ROWS, W], ACC, tag="psA2"
                            )
                            psb2 = pspool.tile(
                                [128, TILE_OUT_ROWS, W], ACC, tag="psB2"
                            )
                        for tap in range(9):
                            kh, kw = divmod(tap, 3)
                            first, last = tap == 0, tap == 8
                            ra = ch[0:64, r0 + kh : r0 + kh + TILE_OUT_ROWS, kw : kw + W]
                            rb = ch[64:128, r0 + kh : r0 + kh + TILE_OUT_ROWS, kw : kw + W]
                            if QUAD:
                                nc.tensor.matmul(
                                    psa[0:64], w_sb[0:64, tap, 0:64], ra,
                                    start=first, stop=last)
                                nc.tensor.matmul(
                                    psb[0:64], w_sb[64:128, tap, 0:64], rb,
                                    start=first, stop=last)
                                nc.tensor.matmul(
                                    psa2[64:128], w_sb[0:64, tap, 64:128], ra,
                                    start=first, stop=last)
                                nc.tensor.matmul(
                                    psb2[64:128], w_sb[64:128, tap, 64:128], rb,
                                    start=first, stop=last)
                            else:
                                nc.tensor.matmul(
                                    psa[:], w_sb[0:64, tap], ra,
                                    start=first, stop=last)
                                nc.tensor.matmul(
                                    psb[:], w_sb[64:128, tap], rb,
                                    start=first, stop=last)
                        if SKIP_EVAC:
                            continue
                        if STAGED_OUT:
                            sl = slice(r0, r0 + TILE_OUT_ROWS)
                            nc.vector.tensor_scalar_add(
                                stage[:, 0, sl], psa[:], b_sb[:])
                            nc.scalar.add(stage[:, 1, sl], psb[:], b_sb[:])
                            continue
                        oa = opool.tile([128, TILE_OUT_ROWS, W], OUT_DT, tag="oA")
                        obt = opool.tile([128, TILE_OUT_ROWS, W], OUT_DT, tag="oB")
                        if QUAD:
                            # Split the 4 half-partition evacuations across
                            # the two PSUM-capable engines.
                            nc.vector.tensor_scalar_add(
                                oa[0:64], psa[0:64], b_sb[0:64])
                            nc.vector.tensor_scalar_add(
                                oa[64:128], psa2[64:128], b_sb[64:128])
                            nc.scalar.add(
                                obt[0:64], psb[0:64], b_sb[0:64])
                            nc.scalar.add(
                                obt[64:128], psb2[64:128], b_sb[64:128])
                        else:
                            # PSUM->SBUF evacuation (+bias) split between
                            # the two PSUM-capable elementwise engines
                            # (GPSIMD/Pool cannot read PSUM on TRN2).
                            nc.vector.tensor_scalar_add(oa[:], psa[:], b_sb[:])
                            nc.scalar.add(obt[:], psb[:], b_sb[:])
                        if SKIP_OUT_DMA:
                            continue
                        orr = orow + r0
                        nc.sync.dma_start(
                            od[:, 0, orr : orr + TILE_OUT_ROWS, :], oa[:]
                        )
                        nc.sync.dma_start(
                            od[:, 1, orr : orr + TILE_OUT_ROWS, :], obt[:]
                        )
                    if STAGED_OUT and not SKIP_EVAC and not SKIP_OUT_DMA:
                        # Write the stage in halves: the first half's DMA
                        # fires as soon as its 8 evacuations complete,
                        # shortening the single-shot tail drain.
                        hr = CHUNK_OUT_ROWS // 2
                        for h in (0, 1):
                            nc.sync.dma_start(
                                od[:, h, orow : orow + hr, :],
                                stage[:, h, 0:hr],
                            )
                            nc.sync.dma_start(
                                od[:, h, orow + hr : orow + CHUNK_OUT_ROWS, :],
                                stage[:, h, hr:CHUNK_OUT_ROWS],
                            )

            body = emit_body_grouped if GROUPED else emit_body
            if KERNEL_REPS > 1:
                with tc.For_i(0, KERNEL_REPS, 1) as _i:
                    body()
            else:
                body()
    nc.finalize()
    return nc


def _get_program():
    global _PROGRAM
    if _PROGRAM is None:
        _PROGRAM = _build_program()
    return _PROGRAM


def _stream_parts(s):
    """Stream s covers full images 2s, 2s+1 and quarter (s%4) of image 32+(s//4)...
    returns (img0, img1, img_q, q) with quarter rows [32q, 32q+32)."""
    img_q = 32 + (s % 4)
    q = s // 4
    return 2 * s, 2 * s + 1, img_q, q


def _make_slab(X, s):
    """Build padded slab [CIN, RSLAB, WP] for stream s from X [NIMG,CIN,H,W]."""
    i0, i1, iq, q = _stream_parts(s)
    sl = np.zeros((CIN, RSLAB, WP), np.float32)
    sl[:, 1 : H + 1, 1 : W + 1] = X[i0]
    sl[:, 131 : 131 + H, 1 : W + 1] = X[i1]
    r0 = 32 * q
    lo, hi = max(r0 - 1, 0), min(r0 + 33, H)
    d0 = 260 + (lo - (r0 - 1))
    sl[:, d0 : d0 + (hi - lo), 1 : W + 1] = X[iq, :, lo:hi]
    return sl


def make_in_maps(x, weight, bias):
    x = np.ascontiguousarray(np.asarray(x), dtype=np.float32)
    weight = np.ascontiguousarray(np.asarray(weight), dtype=np.float32)
    bias = np.ascontiguousarray(np.asarray(bias), dtype=np.float32)
    X = x.reshape(NIMG, CIN, H, W)

    wt = np.ascontiguousarray(weight.transpose(1, 2, 3, 0).reshape(CIN, 9, COUT))
    wt2 = np.ascontiguousarray(np.concatenate([wt, wt], axis=0))  # [128, 9, COUT]
    wt2 = wt2.astype(mybir.dt.np(WEIGHT_DT or DT))
    bb = np.ascontiguousarray(bias.reshape(COUT, 1))

    in_dt = mybir.dt.np(DT)
    in_maps = []
    for c in range(NCORES):
        xs = np.concatenate([_make_slab(X, 2 * c), _make_slab(X, 2 * c + 1)], axis=0)
        in_maps.append(
            {"xs": np.ascontiguousarray(xs.astype(in_dt)), "wt": wt2, "bias": bb}
        )
    return in_maps


def kernel(x, weight, bias):
    in_maps = make_in_maps(x, weight, bias)
    nc = _get_program()
    res = run_bass_kernel_spmd(nc, in_maps, core_ids=list(range(NCORES)))

    Y = np.empty((NIMG, COUT, H, W), np.float32)
    for c in range(NCORES):
        o = res.results[c]["out"].astype(np.float32)  # [COUT, 2, 288, W]
        for half in (0, 1):
            s = 2 * c + half
            i0, i1, iq, q = _stream_parts(s)
            oo = o[:, half]
            Y[i0] = oo[:, 0:H]
            Y[i1] = oo[:, H : 2 * H]
            Y[iq, :, 32 * q : 32 * q + 32, :] = oo[:, 2 * H : 2 * H + 32]
    return Y.reshape(B, PP, COUT, H, W)

